# revision 5
# baseline (speedup 1.0000x reference)
"""LMS adaptive filter (BaseFilter) on 8 TRN2 NeuronCores.

Algorithm per (batch b, frame f): 64-tap LMS over 416 sequential steps.
  e_t   = d[b, 256f + 32 + t] - sum_k w[k] * x[256f + t + k]
  w     = clip(w + MU * e_t * x[256f + t : +64], +-65535)
The clip is essential: mu*|x_win|^2 ~ 3.2 > 2 makes the recursion
unstable, so w rides the clip rails and the rails keep all float
implementations shadowing each other. f32 required (bf16 diverges).

Sharding: 4096 frames split 512/core (both batches on every core) ->
1024 independent sequences/core = 8 groups x 128 partitions, organized
as 2 quads (quad q = batch q, slabs 0-3).

V3: everything except the clip runs on the Vector engine, as two
phase-shifted quad units per step:
  unit q: [4x dot STT+accum -> NSQ slices] [e: 1x TT4 d_t + ns]
          [TMP: TT256 mu*x-window x e-bcast] [W': TT256 add]
  Pool:   clip W' in place (TS256), covered by the other quad's unit
This removes Act from the chain (e is same-engine, in-order -> no
semaphore) and the only cross-engine edge is Pool-clip -> next dots,
hidden by ~1.3us of other-quad work.  d_est is not computed on-chip:
host does d_est = d - e (exact: reference defines e = d - d_est).
"""

import numpy as np

HOP = 256
FRAMELEN = 512
K = 64
WD = 32
MU = 0.05
WMIN, WMAX = -65535.0, 65535.0
B = 2
F = 4096
NC = 8
F_LOC = F // NC              # 512 frames per core
S = (FRAMELEN - K) - WD      # 416 sequential steps
TSTART = (FRAMELEN - HOP) - WD  # 224: first step kept for frames >= 1
TAIL = S - TSTART            # 192 output elements per frame >= 1
SPAN = HOP * (F_LOC - 1) + FRAMELEN  # 131328: x/d elements per core shard
CORE_STRIDE = HOP * F_LOC    # 131072
OUT_LEN = (FRAMELEN - K) + (F - 1) * TAIL  # 786688

NBUF = 2                     # NSQ/PROD buffer depth
DOT_MODE = "narrow"          # "narrow": 4x STT+accum; "quad": TT256+RED4
UNIT_PACE_MS = 0.002         # virtual-time ladder per (t, q) unit

_CACHE = {}


def _build():
    import concourse.bacc as bacc
    import concourse.tile as tile
    from concourse import mybir
    import concourse.bass as bass

    f32 = mybir.dt.float32
    AluOp = mybir.AluOpType

    nc = bacc.Bacc("TRN2", target_bir_lowering=False)
    x_in = nc.dram_tensor("x", [SPAN], f32, kind="ExternalInput")
    d_in = nc.dram_tensor("d", [B, SPAN], f32, kind="ExternalInput")
    # e only; d_est = d - e on host.  [b][f_local][j], j <-> t = TSTART + j
    out_e = nc.dram_tensor("out_e", [B, F_LOC, TAIL], f32,
                           kind="ExternalOutput")
    out_head = nc.dram_tensor("out_head", [B, TSTART], f32,
                              kind="ExternalOutput")

    with tile.TileContext(nc) as tc:
        with tc.tile_pool(name="p", bufs=1) as pool:
            XF = pool.tile([128, 4, FRAMELEN], f32)    # x frames (slab fg)
            XFMU = pool.tile([128, 4, FRAMELEN], f32)  # MU * x frames
            DB = pool.tile([128, B, 4, S], f32)        # d at step offsets
            # weights: quad q covers groups 4q..4q+3 (batch q, slabs 0-3)
            WQ = [[pool.tile([128, 4, K], f32, name=f"WQ{q}_{i}",
                             tag=f"wq{q}_{i}") for i in range(2)]
                  for q in range(2)]
            TMPQ = [[pool.tile([128, 4, K], f32, name=f"TMP{q}_{i}",
                               tag=f"tmp{q}_{i}") for i in range(2)]
                    for q in range(2)]
            # e history: ET[:, t, g]  (serves e-bcast reads AND output)
            ET = pool.tile([128, S, 8], f32, name="ET", tag="et")
            NSQ = [[pool.tile([128, 4], f32, name=f"NSQ{q}_{i}",
                              tag=f"n{q}_{i}") for i in range(NBUF)]
                   for q in range(2)]
            PROD = [[pool.tile([128, 4, K], f32, name=f"PROD{q}_{i}",
                               tag=f"p{q}_{i}") for i in range(NBUF)]
                    for q in range(2)]
            EOUT = pool.tile([128, 8, TAIL], f32, name="EOUT", tag="eout")

            # partition p, slab fg  ->  frame f_local = fg*128 + p
            for q in range(2):
                nc.vector.memset(WQ[q][0][:], 0.0)
            for fg in range(4):
                nc.sync.dma_start(
                    XF[:, fg, :],
                    bass.AP(tensor=x_in, offset=HOP * 128 * fg,
                            ap=[[HOP, 128], [1, FRAMELEN]]),
                )
                nc.vector.tensor_scalar_mul(XFMU[:, fg, :], XF[:, fg, :], MU)
                for b in range(B):
                    nc.sync.dma_start(
                        DB[:, b, fg, :],
                        bass.AP(tensor=d_in,
                                offset=b * SPAN + HOP * 128 * fg + WD,
                                ap=[[HOP, 128], [1, S]]),
                    )

            def xquad(base, t):
                xa = base[:]
                return bass.AP(tensor=xa.tensor, offset=xa.offset + t,
                               ap=[list(xa.ap[0]), [FRAMELEN, 4], [1, K]])

            def emit_unit(t, q):
                i = t % NBUF
                cur, nxt = WQ[q][t % 2], WQ[q][(t + 1) % 2]
                # dots
                if DOT_MODE == "narrow":
                    for j in range(4):
                        nc.vector.scalar_tensor_tensor(
                            out=PROD[q][i][:, j, :],
                            in0=cur[:, j, :], scalar=-1.0,
                            in1=XF[:, j, t:t + K],
                            op0=AluOp.mult, op1=AluOp.mult,
                            accum_out=NSQ[q][i][:, j:j + 1],
                        )
                else:
                    nc.vector.tensor_tensor(
                        out=PROD[q][i][:], in0=cur[:], in1=xquad(XF, t),
                        op=AluOp.mult)
                    nc.vector.tensor_reduce(
                        out=NSQ[q][i][:], in_=PROD[q][i][:],
                        axis=mybir.AxisListType.X, op=AluOp.add,
                        negate=True)
                # e (same engine, in-order: no cross-engine latency)
                da = DB[:]
                dt = bass.AP(tensor=da.tensor,
                             offset=da.offset + (q * 4) * S + t,
                             ap=[list(da.ap[0]), [S, 4]])
                nc.vector.tensor_tensor(
                    out=ET[:, t, 4 * q:4 * q + 4], in0=dt,
                    in1=NSQ[q][i][:], op=AluOp.add)
                # update: TMP = (mu x) * e_bcast ; W' = W + TMP
                ea = ET[:]
                ebc = bass.AP(tensor=ea.tensor,
                              offset=ea.offset + 8 * t + 4 * q,
                              ap=[list(ea.ap[0]), [1, 4], [0, K]])
                tmp = TMPQ[q][t % 2]
                nc.vector.tensor_tensor(out=tmp[:], in0=xquad(XFMU, t),
                                        in1=ebc, op=AluOp.mult)
                nc.vector.tensor_tensor(out=nxt[:], in0=cur[:], in1=tmp[:],
                                        op=AluOp.add)
                # clip on Pool, covered by the other quad's unit
                nc.gpsimd.tensor_scalar(
                    out=nxt[:], in0=nxt[:],
                    scalar1=WMAX, scalar2=WMIN,
                    op0=AluOp.min, op1=AluOp.max,
                )

            # Pin the scheduler to strict unit-sequential order: the legacy
            # CoreSim-based scheduler otherwise hoists the next quad's first
            # dot ahead of this quad's TMP/ADD, exposing the Pool-clip
            # latency as an ~800ns/step stall on the in-order Vector queue.
            # tile_wait_until is virtual (scheduling-sim) time.
            for t in range(S):
                for q in range(2):
                    with tc.tile_wait_until((2 * t + q) * UNIT_PACE_MS):
                        emit_unit(t, q)

            # repack e history (stride-8) into contiguous EOUT, then one DMA
            for g in range(8):
                ea = ET[:]
                src = bass.AP(tensor=ea.tensor,
                              offset=ea.offset + 8 * TSTART + g,
                              ap=[list(ea.ap[0]), [8, TAIL]])
                nc.vector.tensor_copy(out=EOUT[:, g, :], in_=src)
            ea = EOUT[:]
            nc.sync.dma_start(
                bass.AP(tensor=out_e, offset=0,
                        ap=[[TAIL, 128], [F_LOC * TAIL, B], [128 * TAIL, 4],
                            [1, TAIL]]),
                bass.AP(tensor=ea.tensor, offset=ea.offset,
                        ap=[list(ea.ap[0]), [4 * TAIL, B], [TAIL, 4],
                            [1, TAIL]]),
            )
            # head: frame 0 of this core (only core 0's matters), groups b*4
            eh = ET[:]
            for b in range(B):
                nc.sync.dma_start(
                    bass.AP(tensor=out_head, offset=b * TSTART,
                            ap=[[TSTART, 1], [1, TSTART]]),
                    bass.AP(tensor=eh.tensor, offset=eh.offset + 4 * b,
                            ap=[[eh.ap[0][0], 1], [8, TSTART]]),
                )
    nc.finalize()
    return nc


def _get_nc():
    if "nc" not in _CACHE:
        _CACHE["nc"] = _build()
    return _CACHE["nc"]


def run_shards(d, x, trace=False, **kw):
    from concourse.bass_utils import run_bass_kernel_spmd

    nc = _get_nc()
    in_maps = []
    for c in range(NC):
        lo = c * CORE_STRIDE
        in_maps.append({
            "x": np.ascontiguousarray(x[lo:lo + SPAN], dtype=np.float32),
            "d": np.ascontiguousarray(d[:, lo:lo + SPAN], dtype=np.float32),
        })
    return run_bass_kernel_spmd(nc, in_maps, core_ids=list(range(NC)),
                                trace=trace, **kw)


def assemble(results, d):
    es = np.stack([r["out_e"] for r in results])     # (8, B, 512, 192)
    head = results[0]["out_head"]                    # (B, 224)

    # d windows: dwin[b, f, t] = d[b, 256 f + WD + t], t in [0, S)
    idx = HOP * np.arange(F)[:, None] + WD + np.arange(S)[None, :]
    dwin = d[:, idx]                                 # (B, F, S)

    def ola(head_v, main_v):
        # head_v: (B, TSTART) frame-0 steps t<TSTART
        # main_v: (B, F, TAIL) steps t in [TSTART, S) for every frame
        o = np.zeros((B, OUT_LEN), np.float32)
        o[:, WD:WD + TSTART] = head_v
        o[:, WD + TSTART:FRAMELEN - K] = main_v[:, 0]
        o[:, FRAMELEN - K:] = main_v[:, 1:].reshape(B, -1)
        return o

    e_main = es.transpose(1, 0, 2, 3).reshape(B, F, TAIL)
    e_out = ola(head, e_main)
    dest_out = ola(dwin[:, 0, :TSTART] - head,
                   dwin[:, :, TSTART:] - e_main)
    return dest_out, e_out


def kernel(d, x):
    d = np.asarray(d, dtype=np.float32)
    x = np.asarray(x, dtype=np.float32)
    res = run_shards(d, x)
    return assemble(res.results, d)


# revision 6
# speedup vs baseline: 1.1038x; 1.1038x over previous
"""LMS adaptive filter (BaseFilter) on 8 TRN2 NeuronCores.

Algorithm per (batch b, frame f): 64-tap LMS over 416 sequential steps.
  e_t   = d[b, 256f + 32 + t] - sum_k w[k] * x[256f + t + k]
  w     = clip(w + MU * e_t * x[256f + t : +64], +-65535)
The clip is essential: mu*|x_win|^2 ~ 3.2 > 2 makes the recursion
unstable, so w rides the clip rails and the rails keep all float
implementations shadowing each other. f32 required (bf16 diverges).

Sharding: 4096 frames split 512/core (both batches on every core) ->
1024 independent sequences/core = 8 groups x 128 partitions.

V5: the ENTIRE hot loop runs on the Vector engine, single in-order
stream, no other engine touches SBUF meanwhile.  Rationale (measured):
GpSimd shares an SBUF port with DVE's second read port, so any Pool op
overlapping a 2-source DVE op stalls it ("one fully blocks") -- this
inflated every earlier variant ~25%.  With one engine there are no
cross-engine edges, no semaphore stalls, and no scheduler hazards, so
wide ops can be merged across all 8 groups:
  8x dot   : scalar_tensor_tensor + accum -> NS8 slices   (~145ns ea)
  1x e     : TT [128,8]  ET[:,t,:] = d_t + ns             (~77ns)
  1x TMP   : TT [128,512] (mu x-window) * e-bcast         (~602ns)
  1x W'    : TT [128,512] W + TMP                         (~602ns)
  1x clip  : TS [128,512] min/max, 2x perf mode           (~335ns)
d_est is not computed on-chip: host does d_est = d - e (exact, since
the reference defines e = d - d_est).
"""

import numpy as np

HOP = 256
FRAMELEN = 512
K = 64
WD = 32
MU = 0.05
WMIN, WMAX = -65535.0, 65535.0
B = 2
F = 4096
NC = 8
F_LOC = F // NC              # 512 frames per core
S = (FRAMELEN - K) - WD      # 416 sequential steps
TSTART = (FRAMELEN - HOP) - WD  # 224: first step kept for frames >= 1
TAIL = S - TSTART            # 192 output elements per frame >= 1
SPAN = HOP * (F_LOC - 1) + FRAMELEN  # 131328: x/d elements per core shard
CORE_STRIDE = HOP * F_LOC    # 131072
OUT_LEN = (FRAMELEN - K) + (F - 1) * TAIL  # 786688

NBUF = 2                     # NS8/PROD buffer depth

_CACHE = {}


def _build():
    import concourse.bacc as bacc
    import concourse.tile as tile
    from concourse import mybir
    import concourse.bass as bass

    f32 = mybir.dt.float32
    AluOp = mybir.AluOpType

    nc = bacc.Bacc("TRN2", target_bir_lowering=False)
    x_in = nc.dram_tensor("x", [SPAN], f32, kind="ExternalInput")
    d_in = nc.dram_tensor("d", [B, SPAN], f32, kind="ExternalInput")
    # e only; d_est = d - e on host.  [b][f_local][j], j <-> t = TSTART + j
    out_e = nc.dram_tensor("out_e", [B, F_LOC, TAIL], f32,
                           kind="ExternalOutput")
    out_head = nc.dram_tensor("out_head", [B, TSTART], f32,
                              kind="ExternalOutput")

    with tile.TileContext(nc) as tc:
        with tc.tile_pool(name="p", bufs=1) as pool:
            XF = pool.tile([128, 4, FRAMELEN], f32)    # x frames (slab fg)
            XFMU = pool.tile([128, 4, FRAMELEN], f32)  # MU * x frames
            DB = pool.tile([128, B, 4, S], f32)        # d at step offsets
            # all 8 groups' weights in one tile: W[:, g, :], g = 4b + fg
            WALL = [pool.tile([128, 8, K], f32, name=f"WALL{i}",
                              tag=f"wall{i}") for i in range(2)]
            TMP = [pool.tile([128, 8, K], f32, name=f"TMPALL{i}",
                             tag=f"tmpall{i}") for i in range(2)]
            # e history: ET[:, t, g]  (serves e-bcast reads AND output)
            ET = pool.tile([128, S, 8], f32, name="ET", tag="et")
            NS8 = [pool.tile([128, 8], f32, name=f"NS8_{i}", tag=f"n{i}")
                   for i in range(NBUF)]
            PROD = [pool.tile([128, 8, K], f32, name=f"PROD{i}",
                              tag=f"p{i}") for i in range(NBUF)]
            EOUT = pool.tile([128, 8, TAIL], f32, name="EOUT", tag="eout")

            # partition p, slab fg  ->  frame f_local = fg*128 + p
            nc.vector.memset(WALL[0][:], 0.0)
            for fg in range(4):
                nc.sync.dma_start(
                    XF[:, fg, :],
                    bass.AP(tensor=x_in, offset=HOP * 128 * fg,
                            ap=[[HOP, 128], [1, FRAMELEN]]),
                )
                nc.vector.tensor_scalar_mul(XFMU[:, fg, :], XF[:, fg, :], MU)
                for b in range(B):
                    nc.sync.dma_start(
                        DB[:, b, fg, :],
                        bass.AP(tensor=d_in,
                                offset=b * SPAN + HOP * 128 * fg + WD,
                                ap=[[HOP, 128], [1, S]]),
                    )

            for t in range(S):
                i = t % NBUF
                cur, nxt = WALL[t % 2], WALL[(t + 1) % 2]
                tmp = TMP[t % 2]
                # dots: group g = 4b + fg uses x slab fg = g % 4
                for g in range(8):
                    nc.vector.scalar_tensor_tensor(
                        out=PROD[i][:, g, :], in0=cur[:, g, :],
                        scalar=-1.0, in1=XF[:, g % 4, t:t + K],
                        op0=AluOp.mult, op1=AluOp.mult,
                        accum_out=NS8[i][:, g:g + 1],
                    )
                # e for all 8 groups: ET[:, t, :] = d_t + ns
                da = DB[:]
                dt = bass.AP(tensor=da.tensor, offset=da.offset + t,
                             ap=[list(da.ap[0]), [4 * S, B], [S, 4]])
                nc.vector.tensor_tensor(out=ET[:, t, :], in0=dt,
                                        in1=NS8[i][:], op=AluOp.add)
                # update all 8 groups: TMP = (mu x-window) * e_bcast
                xa = XFMU[:]
                xall = bass.AP(tensor=xa.tensor, offset=xa.offset + t,
                               ap=[list(xa.ap[0]), [0, 2], [FRAMELEN, 4],
                                   [1, K]])
                ea = ET[:]
                ebc = bass.AP(tensor=ea.tensor, offset=ea.offset + 8 * t,
                              ap=[list(ea.ap[0]), [1, 8], [0, K]])
                nc.vector.tensor_tensor(out=tmp[:], in0=xall, in1=ebc,
                                        op=AluOp.mult)
                nc.vector.tensor_tensor(out=nxt[:], in0=cur[:], in1=tmp[:],
                                        op=AluOp.add)
                # clip in place, 2x perf mode tensor_scalar on Vector
                nc.vector.tensor_scalar(
                    out=nxt[:], in0=nxt[:],
                    scalar1=WMAX, scalar2=WMIN,
                    op0=AluOp.min, op1=AluOp.max,
                )

            # repack e history (stride-8) into contiguous EOUT, then one DMA
            for g in range(8):
                ea = ET[:]
                src = bass.AP(tensor=ea.tensor,
                              offset=ea.offset + 8 * TSTART + g,
                              ap=[list(ea.ap[0]), [8, TAIL]])
                nc.vector.tensor_copy(out=EOUT[:, g, :], in_=src)
            ea = EOUT[:]
            nc.sync.dma_start(
                bass.AP(tensor=out_e, offset=0,
                        ap=[[TAIL, 128], [F_LOC * TAIL, B], [128 * TAIL, 4],
                            [1, TAIL]]),
                bass.AP(tensor=ea.tensor, offset=ea.offset,
                        ap=[list(ea.ap[0]), [4 * TAIL, B], [TAIL, 4],
                            [1, TAIL]]),
            )
            # head: frame 0 of this core (only core 0's matters), groups b*4
            eh = ET[:]
            for b in range(B):
                nc.sync.dma_start(
                    bass.AP(tensor=out_head, offset=b * TSTART,
                            ap=[[TSTART, 1], [1, TSTART]]),
                    bass.AP(tensor=eh.tensor, offset=eh.offset + 4 * b,
                            ap=[[eh.ap[0][0], 1], [8, TSTART]]),
                )
    nc.finalize()
    return nc


def _get_nc():
    if "nc" not in _CACHE:
        _CACHE["nc"] = _build()
    return _CACHE["nc"]


def run_shards(d, x, trace=False, **kw):
    from concourse.bass_utils import run_bass_kernel_spmd

    nc = _get_nc()
    in_maps = []
    for c in range(NC):
        lo = c * CORE_STRIDE
        in_maps.append({
            "x": np.ascontiguousarray(x[lo:lo + SPAN], dtype=np.float32),
            "d": np.ascontiguousarray(d[:, lo:lo + SPAN], dtype=np.float32),
        })
    return run_bass_kernel_spmd(nc, in_maps, core_ids=list(range(NC)),
                                trace=trace, **kw)


def assemble(results, d):
    es = np.stack([r["out_e"] for r in results])     # (8, B, 512, 192)
    head = results[0]["out_head"]                    # (B, 224)

    # d windows: dwin[b, f, t] = d[b, 256 f + WD + t], t in [0, S)
    idx = HOP * np.arange(F)[:, None] + WD + np.arange(S)[None, :]
    dwin = d[:, idx]                                 # (B, F, S)

    def ola(head_v, main_v):
        # head_v: (B, TSTART) frame-0 steps t<TSTART
        # main_v: (B, F, TAIL) steps t in [TSTART, S) for every frame
        o = np.zeros((B, OUT_LEN), np.float32)
        o[:, WD:WD + TSTART] = head_v
        o[:, WD + TSTART:FRAMELEN - K] = main_v[:, 0]
        o[:, FRAMELEN - K:] = main_v[:, 1:].reshape(B, -1)
        return o

    e_main = es.transpose(1, 0, 2, 3).reshape(B, F, TAIL)
    e_out = ola(head, e_main)
    dest_out = ola(dwin[:, 0, :TSTART] - head,
                   dwin[:, :, TSTART:] - e_main)
    return dest_out, e_out


def kernel(d, x):
    d = np.asarray(d, dtype=np.float32)
    x = np.asarray(x, dtype=np.float32)
    res = run_shards(d, x)
    return assemble(res.results, d)


# revision 8
# speedup vs baseline: 1.1589x; 1.0499x over previous
"""LMS adaptive filter (BaseFilter) on 8 TRN2 NeuronCores.

Algorithm per (batch b, frame f): 64-tap LMS over 416 sequential steps.
  e_t   = d[b, 256f + 32 + t] - sum_k w[k] * x[256f + t + k]
  w     = clip(w + MU * e_t * x[256f + t : +64], +-65535)
The clip is essential: mu*|x_win|^2 ~ 3.2 > 2 makes the recursion
unstable, so w rides the clip rails and the rails keep all float
implementations shadowing each other. f32 required (bf16 diverges).

Sharding: 4096 frames split 512/core (both batches on every core) ->
1024 independent sequences/core = 8 groups x 128 partitions.

V5: the ENTIRE hot loop runs on the Vector engine, single in-order
stream, no other engine touches SBUF meanwhile.  Rationale (measured):
GpSimd shares an SBUF port with DVE's second read port, so any Pool op
overlapping a 2-source DVE op stalls it ("one fully blocks") -- this
inflated every earlier variant ~25%.  With one engine there are no
cross-engine edges, no semaphore stalls, and no scheduler hazards, so
wide ops can be merged across all 8 groups:
  8x dot   : scalar_tensor_tensor + accum -> NS8 slices   (~145ns ea)
  1x e     : TT [128,8]  ET[:,t,:] = d_t + ns             (~77ns)
  1x TMP   : TT [128,512] (mu x-window) * e-bcast         (~602ns)
  1x W'    : TT [128,512] W + TMP                         (~602ns)
  1x clip  : TS [128,512] min/max, 2x perf mode           (~335ns)
d_est is not computed on-chip: host does d_est = d - e (exact, since
the reference defines e = d - d_est).
"""

import numpy as np

HOP = 256
FRAMELEN = 512
K = 64
WD = 32
MU = 0.05
WMIN, WMAX = -65535.0, 65535.0
B = 2
F = 4096
NC = 8
F_LOC = F // NC              # 512 frames per core
S = (FRAMELEN - K) - WD      # 416 sequential steps
TSTART = (FRAMELEN - HOP) - WD  # 224: first step kept for frames >= 1
TAIL = S - TSTART            # 192 output elements per frame >= 1
SPAN = HOP * (F_LOC - 1) + FRAMELEN  # 131328: x/d elements per core shard
CORE_STRIDE = HOP * F_LOC    # 131072
OUT_LEN = (FRAMELEN - K) + (F - 1) * TAIL  # 786688

NBUF = 2                     # NS8/PROD buffer depth

_CACHE = {}


def _build():
    import concourse.bacc as bacc
    import concourse.tile as tile
    from concourse import mybir
    import concourse.bass as bass

    f32 = mybir.dt.float32
    AluOp = mybir.AluOpType

    nc = bacc.Bacc("TRN2", target_bir_lowering=False)
    x_in = nc.dram_tensor("x", [SPAN], f32, kind="ExternalInput")
    d_in = nc.dram_tensor("d", [B, SPAN], f32, kind="ExternalInput")
    # e only; d_est = d - e on host.  [b][f_local][j], j <-> t = TSTART + j
    out_e = nc.dram_tensor("out_e", [B, F_LOC, TAIL], f32,
                           kind="ExternalOutput")
    out_head = nc.dram_tensor("out_head", [B, TSTART], f32,
                              kind="ExternalOutput")

    with tile.TileContext(nc) as tc:
        with tc.tile_pool(name="p", bufs=1) as pool:
            XF = pool.tile([128, 4, FRAMELEN], f32)    # x frames (slab fg)
            XFMU = pool.tile([128, 4, FRAMELEN], f32)  # MU * x frames
            DB = pool.tile([128, B, 4, S], f32)        # d at step offsets
            # all 8 groups' weights in one tile: W[:, g, :], g = 4b + fg
            WALL = [pool.tile([128, 8, K], f32, name=f"WALL{i}",
                              tag=f"wall{i}") for i in range(2)]
            TMP = [pool.tile([128, 8, K], f32, name=f"TMPALL{i}",
                             tag=f"tmpall{i}") for i in range(2)]
            # e history: ET[:, t, g]  (serves e-bcast reads AND output)
            ET = pool.tile([128, S, 8], f32, name="ET", tag="et")
            NS8 = [pool.tile([128, 8], f32, name=f"NS8_{i}", tag=f"n{i}")
                   for i in range(NBUF)]
            PROD = [pool.tile([128, 8, K], f32, name=f"PROD{i}",
                              tag=f"p{i}") for i in range(NBUF)]
            EOUT = pool.tile([128, 8, TAIL], f32, name="EOUT", tag="eout")

            # partition p, slab fg  ->  frame f_local = fg*128 + p
            nc.vector.memset(WALL[0][:], 0.0)
            for fg in range(4):
                nc.sync.dma_start(
                    XF[:, fg, :],
                    bass.AP(tensor=x_in, offset=HOP * 128 * fg,
                            ap=[[HOP, 128], [1, FRAMELEN]]),
                )
                nc.vector.tensor_scalar_mul(XFMU[:, fg, :], XF[:, fg, :], MU)
                for b in range(B):
                    nc.sync.dma_start(
                        DB[:, b, fg, :],
                        bass.AP(tensor=d_in,
                                offset=b * SPAN + HOP * 128 * fg + WD,
                                ap=[[HOP, 128], [1, S]]),
                    )

            for t in range(S):
                i = t % NBUF
                cur, nxt = WALL[t % 2], WALL[(t + 1) % 2]
                tmp = TMP[t % 2]
                # dots: group g = 4b + fg uses x slab fg = g % 4
                for g in range(8):
                    nc.vector.scalar_tensor_tensor(
                        out=PROD[i][:, g, :], in0=cur[:, g, :],
                        scalar=-1.0, in1=XF[:, g % 4, t:t + K],
                        op0=AluOp.mult, op1=AluOp.mult,
                        accum_out=NS8[i][:, g:g + 1],
                    )
                # e for all 8 groups: ET[:, t, :] = d_t + ns
                da = DB[:]
                dt = bass.AP(tensor=da.tensor, offset=da.offset + t,
                             ap=[list(da.ap[0]), [4 * S, B], [S, 4]])
                nc.vector.tensor_tensor(out=ET[:, t, :], in0=dt,
                                        in1=NS8[i][:], op=AluOp.add)
                # update, split in halves: the first half's TMP/ADD/clip
                # unblock the dependent chain ~270ns earlier per leg (RAW
                # edges cost full instruction duration, 157 + FD/accel ns);
                # the second half issues in the first half's shadow.
                xa = XFMU[:]
                ea = ET[:]
                xh = bass.AP(tensor=xa.tensor, offset=xa.offset + t,
                             ap=[list(xa.ap[0]), [FRAMELEN, 4], [1, K]])
                for h in range(2):
                    ebc = bass.AP(tensor=ea.tensor,
                                  offset=ea.offset + 8 * t + 4 * h,
                                  ap=[list(ea.ap[0]), [1, 4], [0, K]])
                    nc.vector.tensor_tensor(out=tmp[:, 4 * h:4 * h + 4, :],
                                            in0=xh, in1=ebc, op=AluOp.mult)
                for h in range(2):
                    nc.vector.tensor_tensor(
                        out=nxt[:, 4 * h:4 * h + 4, :],
                        in0=cur[:, 4 * h:4 * h + 4, :],
                        in1=tmp[:, 4 * h:4 * h + 4, :], op=AluOp.add)
                for h in range(2):
                    nc.vector.tensor_scalar(
                        out=nxt[:, 4 * h:4 * h + 4, :],
                        in0=nxt[:, 4 * h:4 * h + 4, :],
                        scalar1=WMAX, scalar2=WMIN,
                        op0=AluOp.min, op1=AluOp.max,
                    )

            # repack e history (stride-8) into contiguous EOUT, then one DMA
            for g in range(8):
                ea = ET[:]
                src = bass.AP(tensor=ea.tensor,
                              offset=ea.offset + 8 * TSTART + g,
                              ap=[list(ea.ap[0]), [8, TAIL]])
                nc.vector.tensor_copy(out=EOUT[:, g, :], in_=src)
            ea = EOUT[:]
            nc.sync.dma_start(
                bass.AP(tensor=out_e, offset=0,
                        ap=[[TAIL, 128], [F_LOC * TAIL, B], [128 * TAIL, 4],
                            [1, TAIL]]),
                bass.AP(tensor=ea.tensor, offset=ea.offset,
                        ap=[list(ea.ap[0]), [4 * TAIL, B], [TAIL, 4],
                            [1, TAIL]]),
            )
            # head: frame 0 of this core (only core 0's matters), groups b*4
            eh = ET[:]
            for b in range(B):
                nc.sync.dma_start(
                    bass.AP(tensor=out_head, offset=b * TSTART,
                            ap=[[TSTART, 1], [1, TSTART]]),
                    bass.AP(tensor=eh.tensor, offset=eh.offset + 4 * b,
                            ap=[[eh.ap[0][0], 1], [8, TSTART]]),
                )
    nc.finalize()
    return nc


def _get_nc():
    if "nc" not in _CACHE:
        _CACHE["nc"] = _build()
    return _CACHE["nc"]


def run_shards(d, x, trace=False, **kw):
    from concourse.bass_utils import run_bass_kernel_spmd

    nc = _get_nc()
    in_maps = []
    for c in range(NC):
        lo = c * CORE_STRIDE
        in_maps.append({
            "x": np.ascontiguousarray(x[lo:lo + SPAN], dtype=np.float32),
            "d": np.ascontiguousarray(d[:, lo:lo + SPAN], dtype=np.float32),
        })
    return run_bass_kernel_spmd(nc, in_maps, core_ids=list(range(NC)),
                                trace=trace, **kw)


def assemble(results, d):
    es = np.stack([r["out_e"] for r in results])     # (8, B, 512, 192)
    head = results[0]["out_head"]                    # (B, 224)

    # d windows: dwin[b, f, t] = d[b, 256 f + WD + t], t in [0, S)
    idx = HOP * np.arange(F)[:, None] + WD + np.arange(S)[None, :]
    dwin = d[:, idx]                                 # (B, F, S)

    def ola(head_v, main_v):
        # head_v: (B, TSTART) frame-0 steps t<TSTART
        # main_v: (B, F, TAIL) steps t in [TSTART, S) for every frame
        o = np.zeros((B, OUT_LEN), np.float32)
        o[:, WD:WD + TSTART] = head_v
        o[:, WD + TSTART:FRAMELEN - K] = main_v[:, 0]
        o[:, FRAMELEN - K:] = main_v[:, 1:].reshape(B, -1)
        return o

    e_main = es.transpose(1, 0, 2, 3).reshape(B, F, TAIL)
    e_out = ola(head, e_main)
    dest_out = ola(dwin[:, 0, :TSTART] - head,
                   dwin[:, :, TSTART:] - e_main)
    return dest_out, e_out


def kernel(d, x):
    d = np.asarray(d, dtype=np.float32)
    x = np.asarray(x, dtype=np.float32)
    res = run_shards(d, x)
    return assemble(res.results, d)


# revision 9
# speedup vs baseline: 1.2043x; 1.0392x over previous
"""LMS adaptive filter (BaseFilter) on 8 TRN2 NeuronCores.

Algorithm per (batch b, frame f): 64-tap LMS over 416 sequential steps.
  e_t   = d[b, 256f + 32 + t] - sum_k w[k] * x[256f + t + k]
  w     = clip(w + MU * e_t * x[256f + t : +64], +-65535)
The clip is essential: mu*|x_win|^2 ~ 3.2 > 2 makes the recursion
unstable, so w rides the clip rails and the rails keep all float
implementations shadowing each other. f32 required (bf16 diverges).

Sharding: 4096 frames split 512/core (both batches on every core) ->
1024 independent sequences/core = 8 groups x 128 partitions.

V5: the ENTIRE hot loop runs on the Vector engine, single in-order
stream, no other engine touches SBUF meanwhile.  Rationale (measured):
GpSimd shares an SBUF port with DVE's second read port, so any Pool op
overlapping a 2-source DVE op stalls it ("one fully blocks") -- this
inflated every earlier variant ~25%.  With one engine there are no
cross-engine edges, no semaphore stalls, and no scheduler hazards, so
wide ops can be merged across all 8 groups:
  8x dot   : scalar_tensor_tensor + accum -> NS8 slices   (~145ns ea)
  1x e     : TT [128,8]  ET[:,t,:] = d_t + ns             (~77ns)
  1x TMP   : TT [128,512] (mu x-window) * e-bcast         (~602ns)
  1x W'    : TT [128,512] W + TMP                         (~602ns)
  1x clip  : TS [128,512] min/max, 2x perf mode           (~335ns)
d_est is not computed on-chip: host does d_est = d - e (exact, since
the reference defines e = d - d_est).
"""

import numpy as np

HOP = 256
FRAMELEN = 512
K = 64
WD = 32
MU = 0.05
WMIN, WMAX = -65535.0, 65535.0
B = 2
F = 4096
NC = 8
F_LOC = F // NC              # 512 frames per core
S = (FRAMELEN - K) - WD      # 416 sequential steps
TSTART = (FRAMELEN - HOP) - WD  # 224: first step kept for frames >= 1
TAIL = S - TSTART            # 192 output elements per frame >= 1
SPAN = HOP * (F_LOC - 1) + FRAMELEN  # 131328: x/d elements per core shard
CORE_STRIDE = HOP * F_LOC    # 131072
OUT_LEN = (FRAMELEN - K) + (F - 1) * TAIL  # 786688

NBUF = 2                     # NS8/PROD buffer depth

_CACHE = {}


def _build():
    import concourse.bacc as bacc
    import concourse.tile as tile
    from concourse import mybir
    import concourse.bass as bass

    f32 = mybir.dt.float32
    AluOp = mybir.AluOpType

    nc = bacc.Bacc("TRN2", target_bir_lowering=False)
    x_in = nc.dram_tensor("x", [SPAN], f32, kind="ExternalInput")
    d_in = nc.dram_tensor("d", [B, SPAN], f32, kind="ExternalInput")
    # e only; d_est = d - e on host.  [b][f_local][j], j <-> t = TSTART + j
    out_e = nc.dram_tensor("out_e", [B, F_LOC, TAIL], f32,
                           kind="ExternalOutput")
    out_head = nc.dram_tensor("out_head", [B, TSTART], f32,
                              kind="ExternalOutput")

    with tile.TileContext(nc) as tc:
        with tc.tile_pool(name="p", bufs=1) as pool:
            XF = pool.tile([128, 4, FRAMELEN], f32)    # x frames (slab fg)
            XFMU = pool.tile([128, 4, FRAMELEN], f32)  # MU * x frames
            DB = pool.tile([128, B, 4, S], f32)        # d at step offsets
            # all 8 groups' weights in one tile: W[:, g, :], g = 4b + fg
            WALL = [pool.tile([128, 8, K], f32, name=f"WALL{i}",
                              tag=f"wall{i}") for i in range(2)]
            TMP = [pool.tile([128, 8, K], f32, name=f"TMPALL{i}",
                             tag=f"tmpall{i}") for i in range(2)]
            # e history: ET[:, t, g]  (serves e-bcast reads AND output)
            ET = pool.tile([128, S, 8], f32, name="ET", tag="et")
            NS8 = [pool.tile([128, 8], f32, name=f"NS8_{i}", tag=f"n{i}")
                   for i in range(NBUF)]
            PROD = [pool.tile([128, 8, K], f32, name=f"PROD{i}",
                              tag=f"p{i}") for i in range(NBUF)]
            EOUT = pool.tile([128, 8, TAIL], f32, name="EOUT", tag="eout")

            # partition p, slab fg  ->  frame f_local = fg*128 + p
            nc.vector.memset(WALL[0][:], 0.0)
            for fg in range(4):
                nc.sync.dma_start(
                    XF[:, fg, :],
                    bass.AP(tensor=x_in, offset=HOP * 128 * fg,
                            ap=[[HOP, 128], [1, FRAMELEN]]),
                )
                nc.vector.tensor_scalar_mul(XFMU[:, fg, :], XF[:, fg, :], MU)
                for b in range(B):
                    nc.sync.dma_start(
                        DB[:, b, fg, :],
                        bass.AP(tensor=d_in,
                                offset=b * SPAN + HOP * 128 * fg + WD,
                                ap=[[HOP, 128], [1, S]]),
                    )

            for t in range(S):
                i = t % NBUF
                cur, nxt = WALL[t % 2], WALL[(t + 1) % 2]
                tmp = TMP[t % 2]

                def emit_dot(g):
                    nc.vector.scalar_tensor_tensor(
                        out=PROD[i][:, g, :], in0=cur[:, g, :],
                        scalar=-1.0, in1=XF[:, g % 4, t:t + K],
                        op0=AluOp.mult, op1=AluOp.mult,
                        accum_out=NS8[i][:, g:g + 1],
                    )

                def emit_e(h):
                    # e half: ET[:, t, 4h:4h+4] = d_t + ns, issued inside
                    # the dot stream so the accumulator-ack and TT-duration
                    # edges are covered by later dots' issue slots
                    da = DB[:]
                    dt = bass.AP(tensor=da.tensor,
                                 offset=da.offset + h * 4 * S + t,
                                 ap=[list(da.ap[0]), [S, 4]])
                    nc.vector.tensor_tensor(out=ET[:, t, 4 * h:4 * h + 4],
                                            in0=dt,
                                            in1=NS8[i][:, 4 * h:4 * h + 4],
                                            op=AluOp.add)

                for g in range(6):
                    emit_dot(g)
                emit_e(0)       # needs dots 0-3 only
                emit_dot(6)
                emit_dot(7)
                emit_e(1)       # needs dots 4-7
                # update, split in halves: the first half's TMP/ADD/clip
                # unblock the dependent chain ~270ns earlier per leg (RAW
                # edges cost full instruction duration, 157 + FD/accel ns);
                # the second half issues in the first half's shadow.
                xa = XFMU[:]
                ea = ET[:]
                xh = bass.AP(tensor=xa.tensor, offset=xa.offset + t,
                             ap=[list(xa.ap[0]), [FRAMELEN, 4], [1, K]])
                for h in range(2):
                    ebc = bass.AP(tensor=ea.tensor,
                                  offset=ea.offset + 8 * t + 4 * h,
                                  ap=[list(ea.ap[0]), [1, 4], [0, K]])
                    nc.vector.tensor_tensor(out=tmp[:, 4 * h:4 * h + 4, :],
                                            in0=xh, in1=ebc, op=AluOp.mult)
                for h in range(2):
                    nc.vector.tensor_tensor(
                        out=nxt[:, 4 * h:4 * h + 4, :],
                        in0=cur[:, 4 * h:4 * h + 4, :],
                        in1=tmp[:, 4 * h:4 * h + 4, :], op=AluOp.add)
                for h in range(2):
                    nc.vector.tensor_scalar(
                        out=nxt[:, 4 * h:4 * h + 4, :],
                        in0=nxt[:, 4 * h:4 * h + 4, :],
                        scalar1=WMAX, scalar2=WMIN,
                        op0=AluOp.min, op1=AluOp.max,
                    )

            # repack e history (stride-8) into contiguous EOUT, then one DMA
            for g in range(8):
                ea = ET[:]
                src = bass.AP(tensor=ea.tensor,
                              offset=ea.offset + 8 * TSTART + g,
                              ap=[list(ea.ap[0]), [8, TAIL]])
                nc.vector.tensor_copy(out=EOUT[:, g, :], in_=src)
            ea = EOUT[:]
            nc.sync.dma_start(
                bass.AP(tensor=out_e, offset=0,
                        ap=[[TAIL, 128], [F_LOC * TAIL, B], [128 * TAIL, 4],
                            [1, TAIL]]),
                bass.AP(tensor=ea.tensor, offset=ea.offset,
                        ap=[list(ea.ap[0]), [4 * TAIL, B], [TAIL, 4],
                            [1, TAIL]]),
            )
            # head: frame 0 of this core (only core 0's matters), groups b*4
            eh = ET[:]
            for b in range(B):
                nc.sync.dma_start(
                    bass.AP(tensor=out_head, offset=b * TSTART,
                            ap=[[TSTART, 1], [1, TSTART]]),
                    bass.AP(tensor=eh.tensor, offset=eh.offset + 4 * b,
                            ap=[[eh.ap[0][0], 1], [8, TSTART]]),
                )
    nc.finalize()
    return nc


def _get_nc():
    if "nc" not in _CACHE:
        _CACHE["nc"] = _build()
    return _CACHE["nc"]


def run_shards(d, x, trace=False, **kw):
    from concourse.bass_utils import run_bass_kernel_spmd

    nc = _get_nc()
    in_maps = []
    for c in range(NC):
        lo = c * CORE_STRIDE
        in_maps.append({
            "x": np.ascontiguousarray(x[lo:lo + SPAN], dtype=np.float32),
            "d": np.ascontiguousarray(d[:, lo:lo + SPAN], dtype=np.float32),
        })
    return run_bass_kernel_spmd(nc, in_maps, core_ids=list(range(NC)),
                                trace=trace, **kw)


def assemble(results, d):
    es = np.stack([r["out_e"] for r in results])     # (8, B, 512, 192)
    head = results[0]["out_head"]                    # (B, 224)

    # d windows: dwin[b, f, t] = d[b, 256 f + WD + t], t in [0, S)
    idx = HOP * np.arange(F)[:, None] + WD + np.arange(S)[None, :]
    dwin = d[:, idx]                                 # (B, F, S)

    def ola(head_v, main_v):
        # head_v: (B, TSTART) frame-0 steps t<TSTART
        # main_v: (B, F, TAIL) steps t in [TSTART, S) for every frame
        o = np.zeros((B, OUT_LEN), np.float32)
        o[:, WD:WD + TSTART] = head_v
        o[:, WD + TSTART:FRAMELEN - K] = main_v[:, 0]
        o[:, FRAMELEN - K:] = main_v[:, 1:].reshape(B, -1)
        return o

    e_main = es.transpose(1, 0, 2, 3).reshape(B, F, TAIL)
    e_out = ola(head, e_main)
    dest_out = ola(dwin[:, 0, :TSTART] - head,
                   dwin[:, :, TSTART:] - e_main)
    return dest_out, e_out


def kernel(d, x):
    d = np.asarray(d, dtype=np.float32)
    x = np.asarray(x, dtype=np.float32)
    res = run_shards(d, x)
    return assemble(res.results, d)
